# revision 6
# baseline (speedup 1.0000x reference)
"""Multi-head attention (RoPE + masked softmax) on 8 TRN2 NeuronCores.

Sharding: 2 batch groups x 4 cores; each core computes 4 of the 16 heads
(q/k/v projection column slices) for the full sequence of its batch, then a
partial output projection with its 256-row slice of Wo, and a per-window
ReduceScatter(add) over the 4-core group sums the o-proj partials while
handing each core one 128-row seq sub-shard per window (host reassembles).

The attention mask is classified host-side into 128x128 blocks
(skip / identity / mixed) shared across both batches, and the program is
compiled specialized to that block map: fully-masked blocks are never
computed, identity blocks skip the mask multiply, and mixed blocks multiply
a per-block exp(mask) tile. Causal masks dedupe to a single mixed tile.

Schedule notes: inputs are DMA'd in consumption order with fine-grained
tiles so the first projection matmul starts ~2us in; projection half 1 and
the previous window's partial-o-proj chunks are interleaved into the
attention item stream as filler so the PE stays continuously busy (p-state).
"""

from contextlib import ExitStack

import numpy as np

import concourse.bass as bass
import concourse.tile as tile
from concourse import bacc, mybir
from concourse.alu_op_type import AluOpType
from concourse.bass_utils import run_bass_kernel_spmd

AF = mybir.ActivationFunctionType
F32 = mybir.dt.float32
F32R = mybir.dt.float32r
F16 = mybir.dt.float16
BF16 = mybir.dt.bfloat16

B, S, HID, NH, HD = 2, 2048, 1024, 16, 64
SCALE = 1.0 / np.sqrt(HD)
N_CORES = 8
HPC = 4          # heads per core
HC = HID // 128  # hidden 128-chunks (8)
KC = S // 128    # key 128-chunks (16)
QB = S // 512    # query windows of 512 (4)
GROUPS = [[0, 1, 2, 3], [4, 5, 6, 7]]

SKIP = -1
ONES = 0


def _runs(labels):
    """Maximal runs of non-skip entries: [(start, end)]."""
    out = []
    i = 0
    n = len(labels)
    while i < n:
        if labels[i] == SKIP:
            i += 1
            continue
        j = i
        while j < n and labels[j] != SKIP:
            j += 1
        out.append((i, j))
        i = j
    return out


def build_program(cls_map):
    """cls_map[kc][qc] over the 16x16 grid of (key, query) 128x128 blocks:
    SKIP (-1), ONES (0), or 1-based mixed-tile index."""
    n_mixed = max((v for row in cls_map for v in row if v > 0), default=0)

    nc = bacc.Bacc("TRN2", target_bir_lowering=False, debug=False,
                   num_devices=N_CORES)

    hsT = nc.dram_tensor("hsT", [HID, S], BF16, kind="ExternalInput").ap()
    cosk = nc.dram_tensor("cosk", [128, S], BF16, kind="ExternalInput").ap()
    sink = nc.dram_tensor("sink", [128, S], BF16, kind="ExternalInput").ap()
    wq = nc.dram_tensor("wq", [HID, 256], BF16, kind="ExternalInput").ap()
    wk = nc.dram_tensor("wk", [HID, 256], BF16, kind="ExternalInput").ap()
    wv = nc.dram_tensor("wv", [HID, 256], BF16, kind="ExternalInput").ap()
    wo = nc.dram_tensor("wo", [256, HID], BF16, kind="ExternalInput").ap()
    emt = nc.dram_tensor("emt", [128, max(n_mixed, 1) * 128], BF16,
                         kind="ExternalInput").ap()
    sel4 = nc.dram_tensor("sel4", [128, 256], F32R, kind="ExternalInput").ap()
    out = nc.dram_tensor("out", [QB * 128, HID], F16, kind="ExternalOutput").ap()

    with tile.TileContext(nc) as tc, ExitStack() as top:
        res = top.enter_context(tc.tile_pool(name="res", bufs=1))
        dram = top.enter_context(tc.tile_pool(name="dram", bufs=1, space="DRAM"))

        # ---- warm-up collective: first thing on the gpsimd queue ---------
        rs_wu_in = dram.tile([32, 8], F16, name="rswui")
        rs_wu_out = dram.tile([8, 8], F16, name="rswuo")
        nc.gpsimd.collective_compute(
            "ReduceScatter", AluOpType.add, replica_groups=GROUPS,
            ins=[rs_wu_in[:].opt()], outs=[rs_wu_out[:].opt()])

        # ---- resident tiles + input DMAs (consumption order) -------------
        # sync queue: hsT half-tiles in the order projections consume them.
        hsT_sb = [[None, None] for _ in range(HC)]
        for half in range(2):
            for hc in range(HC):
                t = res.tile([128, 1024], BF16, tag=f"hsT{hc}_{half}",
                             name=f"hsT{hc}_{half}")
                nc.sync.dma_start(t[:], hsT[hc * 128:(hc + 1) * 128,
                                            half * 1024:(half + 1) * 1024])
                hsT_sb[hc][half] = t
        # scalar queue: weights + rope tables, first-needed first.
        wk_sb, wq_sb, wv_sb = [], [], []
        cos_sb, sin_sb = [], []

        def w_tiles(nm, src, dst):
            for hc in range(HC):
                t = res.tile([128, 256], BF16, tag=f"{nm}r{hc}",
                             name=f"{nm}r{hc}")
                nc.scalar.dma_start(t[:], src[hc * 128:(hc + 1) * 128, :])
                dst.append(t)

        def rope_tiles(half):
            for nm, src, dst in (("cos", cosk, cos_sb), ("sin", sink, sin_sb)):
                t = res.tile([128, 1024], BF16, tag=f"{nm}{half}",
                             name=f"{nm}{half}")
                nc.scalar.dma_start(
                    t[:], src[:, half * 1024:(half + 1) * 1024])
                dst.append(t)

        w_tiles("wk", wk, wk_sb)
        rope_tiles(0)
        w_tiles("wq", wq, wq_sb)
        w_tiles("wv", wv, wv_sb)
        rope_tiles(1)
        emt_sb = res.tile([128, max(n_mixed, 1) * 128], BF16, tag="emt")
        nc.scalar.dma_start(emt_sb[:], emt[:])
        sel_sb = res.tile([128, 256], F32R, tag="sel")
        nc.scalar.dma_start(sel_sb[:], sel4[:])
        wo_sb = []
        for p in range(2):
            t = res.tile([128, HID], BF16, tag=f"wo{p}", name=f"wo{p}")
            nc.scalar.dma_start(t[:], wo[p * 128:(p + 1) * 128, :])
            wo_sb.append(t)

        # V_aug: per key-chunk, 4 heads x (64 cols + ones col)
        v_sb = [res.tile([128, HPC * 65], BF16, tag=f"v{kc}", name=f"v{kc}")
                for kc in range(KC)]
        for kc in range(KC):
            v3 = v_sb[kc][:].rearrange("p (h c) -> p h c", h=HPC)
            nc.gpsimd.memset(v3[:, :, 64], 1.0)
        kt_sb = [res.tile([128, S], BF16, tag=f"kt{p}", name=f"kt{p}")
                 for p in range(2)]
        qt_sb = [res.tile([128, S], BF16, tag=f"qt{p}", name=f"qt{p}")
                 for p in range(2)]
        # per-window unnormalized / normalized attention outputs
        acc_w = [[res.tile([128, 512], BF16, tag=f"acc{w}_{p}",
                           name=f"acc{w}_{p}")
                  for p in range(2)] for w in range(QB)]
        acc2_w = [[res.tile([128, 512], BF16, tag=f"acc2_{w}_{p}",
                            name=f"acc2_{w}_{p}")
                   for p in range(2)] for w in range(QB)]
        # den rows live at partition 32*h (DVE writes must start 32-aligned)
        den_w = [res.tile([128, 512], F32, tag=f"den{w}", name=f"den{w}")
                 for w in range(QB)]
        recip_w = [res.tile([128, 512], F32R, tag=f"recip{w}",
                            name=f"recip{w}")
                   for w in range(QB)]
        for w in range(QB):
            nc.gpsimd.memset(den_w[w][:], 1.0)

        # DRAM bounce for the per-window o-proj partial ReduceScatter
        rs_in = [dram.tile([512, HID], F16, name=f"rsi{w}") for w in range(QB)]
        rs_out = [dram.tile([128, HID], F16, name=f"rso{w}")
                  for w in range(QB)]

        pss = top.enter_context(tc.tile_pool(name="pss", bufs=2, space="PSUM"))
        psa = top.enter_context(tc.tile_pool(name="psa", bufs=2, space="PSUM"))
        pbo = top.enter_context(tc.tile_pool(name="pbo", bufs=2, space="PSUM"))
        pp = top.enter_context(tc.tile_pool(name="pp", bufs=5))
        outp = top.enter_context(tc.tile_pool(name="outp", bufs=3))
        ropep = top.enter_context(tc.tile_pool(name="ropep", bufs=6))

        # ---- Q/K/V projections + RoPE ------------------------------------
        def rope_apply(ps2, dst_tile, half, copy_eng):
            """dst[:, half*1024:+1024] = rope(ps2) over a full 1024-col chain.
            One psum->sbuf copy (scalar or vector), a DMA partition-shuffle
            for rotate-half, then three all-SBUF gpsimd ops (sin table is
            pre-shuffled/signed host-side)."""
            g0 = half * 1024
            cs = cos_sb[half]
            sn = sin_sb[half]
            with nc.allow_low_precision(reason="bf16 rope"):
                kraw = ropep.tile([128, 1024], BF16, tag="kraw", name="kraw")
                if copy_eng is nc.scalar:
                    nc.scalar.copy(kraw[:], ps2[:])
                else:
                    nc.vector.tensor_copy(kraw[:], ps2[:])
                krot = ropep.tile([128, 1024], BF16, tag="krot", name="krot")
                for hb in (0, 64):
                    nc.gpsimd.dma_start(krot[hb:hb + 32, :],
                                        kraw[hb + 32:hb + 64, :])
                    nc.gpsimd.dma_start(krot[hb + 32:hb + 64, :],
                                        kraw[hb:hb + 32, :])
                t1 = ropep.tile([128, 1024], BF16, tag="t1", name="t1")
                nc.gpsimd.tensor_tensor(t1[:], kraw[:], cs[:],
                                        AluOpType.mult)
                nc.gpsimd.tensor_tensor(krot[:], krot[:], sn[:],
                                        AluOpType.mult)
                nc.gpsimd.tensor_tensor(
                    dst_tile[:, g0:g0 + 1024], t1[:], krot[:], AluOpType.add)

        def emit_kq_chain(w_sb, dst, p, half, copy_eng):
            """One [128, 1024] projection chain + rope for both 512-windows."""
            c0 = p * 128
            ps2 = pss.tile([128, 1024], F32, tag="pss", name="pskq")
            for hc in range(HC):
                for j in range(2):
                    nc.tensor.matmul(
                        ps2[:, j * 512:(j + 1) * 512],
                        w_sb[hc][:, c0:c0 + 128],
                        hsT_sb[hc][half][:, j * 512:(j + 1) * 512],
                        start=(hc == 0), stop=(hc == HC - 1))
            rope_apply(ps2, dst[p], half, copy_eng)

        def emit_v(kc):
            psf = pbo.tile([128, 512], F32, tag="pbo", name="psvf")
            ps = psf[:, 0:256]
            half, cc = kc // 8, (kc % 8) * 128
            for hc in range(HC):
                nc.tensor.matmul(
                    ps, hsT_sb[hc][half][:, cc:cc + 128],
                    wv_sb[hc][:], start=(hc == 0), stop=(hc == HC - 1))
            v3 = v_sb[kc][:].rearrange("p (h c) -> p h c", h=HPC)
            ps3 = ps.rearrange("p (h c) -> p h c", h=HPC)
            with nc.allow_low_precision(reason="bf16 V"):
                nc.vector.tensor_copy(v3[:, :, 0:64], ps3[:])

        def proj_half_units(half):
            """Filler units (thunks) for projecting one 1024-col half."""
            units = []
            units.append(lambda h=half: emit_kq_chain(wk_sb, kt_sb, 0, h,
                                                      nc.scalar))
            units.append(lambda h=half: emit_kq_chain(wk_sb, kt_sb, 1, h,
                                                      nc.vector))
            units.append(lambda h=half: emit_v(8 * half + 0))
            units.append(lambda h=half: emit_v(8 * half + 1))
            units.append(lambda h=half: emit_kq_chain(wq_sb, qt_sb, 0, h,
                                                      nc.scalar))
            units.append(lambda h=half: emit_v(8 * half + 2))
            units.append(lambda h=half: emit_v(8 * half + 3))
            units.append(lambda h=half: emit_kq_chain(wq_sb, qt_sb, 1, h,
                                                      nc.vector))
            for kc in range(8 * half + 4, 8 * half + 8):
                units.append(lambda k=kc: emit_v(k))
            return units

        # ---- attention items ---------------------------------------------
        # item = (h, qb, kcs(list), runs(per kc), start, stop)
        by_window = []
        for qb in range(QB):
            items = []
            for h in range(HPC):
                parts = [kc for kc in range(KC)
                         if any(cls_map[kc][4 * qb + j] != SKIP
                                for j in range(4))]
                assert parts, f"no participating key blocks for window {qb}"
                for i2 in range(0, len(parts), 2):
                    kcs = parts[i2:i2 + 2]
                    runs = [_runs([cls_map[kc][4 * qb + j] for j in range(4)])
                            for kc in kcs]
                    items.append([h, qb, kcs, runs,
                                  i2 == 0, i2 + 2 >= len(parts)])
            by_window.append(items)

        psa_t = {}
        ps_of_item = {}

        def kt_slice(p, hb, kc):
            half, cc = kc // 8, (kc % 8) * 128
            return kt_sb[p][hb:hb + 64, half * 1024 + cc: half * 1024 + cc + 128]

        def emit_scores(it):
            h, qb, kcs, runs, start, stop = it
            p, hb = h // 2, (h % 2) * 64
            ps_s = pss.tile([128, 1024], F32, tag="pss")
            for j, kc in enumerate(kcs):
                for r0, r1 in runs[j]:
                    nc.tensor.matmul(
                        ps_s[:, j * 512 + r0 * 128: j * 512 + r1 * 128],
                        kt_slice(p, hb, kc),
                        qt_sb[p][hb:hb + 64,
                                 qb * 512 + r0 * 128: qb * 512 + r1 * 128],
                        start=True, stop=True)
            ps_of_item[id(it)] = ps_s

        def emit_expmask(it):
            h, qb, kcs, runs, start, stop = it
            ps_s = ps_of_item.pop(id(it))
            P = pp.tile([128, 1024], BF16, tag="pp")
            if len(kcs) == 2 and runs[0] == runs[1] == [(0, 4)]:
                nc.scalar.activation(P[:], ps_s[:], AF.Exp)
            else:
                for j in range(len(kcs)):
                    for r0, r1 in runs[j]:
                        nc.scalar.activation(
                            P[:, j * 512 + r0 * 128: j * 512 + r1 * 128],
                            ps_s[:, j * 512 + r0 * 128: j * 512 + r1 * 128],
                            AF.Exp)
            for j, kc in enumerate(kcs):
                labels = [cls_map[kc][4 * qb + jj] for jj in range(4)]
                if start and j == 0 and runs[j] != [(0, 4)]:
                    # first accumulation must initialize all 512 cols
                    zr = [jj for jj in range(4) if labels[jj] == SKIP]
                    jj = 0
                    while jj < len(zr):
                        j2 = jj
                        while j2 + 1 < len(zr) and zr[j2 + 1] == zr[j2] + 1:
                            j2 += 1
                        nc.gpsimd.memset(
                            P[:, j * 512 + zr[jj] * 128:
                              j * 512 + (zr[j2] + 1) * 128],
                            0.0)
                        jj = j2 + 1
                for jj in range(4):
                    mix = labels[jj]
                    if mix > 0:
                        sl = P[:, j * 512 + jj * 128: j * 512 + (jj + 1) * 128]
                        with nc.allow_low_precision(reason="bf16 mask"):
                            nc.gpsimd.tensor_tensor(
                                sl, sl,
                                emt_sb[:, (mix - 1) * 128: mix * 128],
                                AluOpType.mult)
            return P

        def emit_attnv(it, P):
            h, qb, kcs, runs, start, stop = it
            if start:
                psa_t[(h, qb)] = psa.tile([128, 512], F32, tag="psa",
                                          name="psat")
            pa = psa_t[(h, qb)]
            for j, kc in enumerate(kcs):
                first = start and j == 0
                last = stop and j == len(kcs) - 1
                if first:
                    nc.tensor.matmul(
                        pa[0:65, :], v_sb[kc][:, h * 65: h * 65 + 65],
                        P[:, j * 512:(j + 1) * 512],
                        start=True, stop=last)
                else:
                    for r0, r1 in runs[j]:
                        nc.tensor.matmul(
                            pa[0:65, r0 * 128:r1 * 128],
                            v_sb[kc][:, h * 65: h * 65 + 65],
                            P[:, j * 512 + r0 * 128: j * 512 + r1 * 128],
                            start=False,
                            stop=last and (r0, r1) == runs[j][-1])
            if stop:
                p, hb = h // 2, (h % 2) * 64
                nc.vector.tensor_copy(den_w[qb][32 * h:32 * h + 1, :],
                                      pa[64:65, :])
                with nc.allow_low_precision(reason="bf16 attn accum"):
                    nc.vector.tensor_copy(acc_w[qb][p][hb:hb + 64, :],
                                          pa[0:64, :])
                del psa_t[(h, qb)]

        def emit_window_norm(w):
            from concourse.dve_ops import (
                RECIP_APPROX_FAST_CONSTS,
                RECIPROCAL_APPROX_FAST,
            )
            c = RECIP_APPROX_FAST_CONSTS
            with nc.allow_low_precision(reason="f32r reciprocal"):
                nc.vector._custom_dve(
                    RECIPROCAL_APPROX_FAST, out=recip_w[w][:],
                    in0=den_w[w][:], s0=c["s0"], s1=c["s1"], imm2=c["imm2"])
            for p in range(2):
                ps_bc = pbo.tile([128, 512], F32, tag="pbo", name="psbc")
                nc.tensor.matmul(ps_bc[:],
                                 sel_sb[:, p * 128:(p + 1) * 128],
                                 recip_w[w][:], start=True, stop=True)
                with nc.allow_low_precision(reason="bf16 attn weights"):
                    nc.vector.tensor_tensor(
                        acc2_w[w][p][:], acc_w[w][p][:], ps_bc[:],
                        AluOpType.mult)

        def emit_oproj_chunk(w, qc):
            """Partial o-proj for 128 q rows of window w into rs_in[w]."""
            t_o = outp.tile([128, HID], F16, tag="tout")
            for nn in range(2):
                ps = pbo.tile([128, 512], F32, tag="pbo", name="pso")
                for p in range(2):
                    nc.tensor.matmul(
                        ps[:], acc2_w[w][p][:, qc * 128:(qc + 1) * 128],
                        wo_sb[p][:, nn * 512:(nn + 1) * 512],
                        start=(p == 0), stop=(p == 1))
                with nc.allow_low_precision(reason="f16 out"):
                    nc.vector.tensor_copy(t_o[:, nn * 512:(nn + 1) * 512],
                                          ps[:])
            nc.sync.dma_start(rs_in[w][qc * 128:(qc + 1) * 128, :], t_o[:])

        def emit_rs(w):
            nc.gpsimd.collective_compute(
                "ReduceScatter", AluOpType.add, replica_groups=GROUPS,
                ins=[rs_in[w][:].opt()],
                outs=[rs_out[w][:].opt()])
            nc.sync.dma_start(out[w * 128:(w + 1) * 128, :], rs_out[w][:])

        # ---- schedule -----------------------------------------------------
        # proj half 0 up front; proj half 1 + previous window's o-proj
        # chunks dispensed as filler between attention items.
        for unit in proj_half_units(0):
            unit()

        def window_fillers(w):
            if w == 0:
                return []
            if w == 1:
                return proj_half_units(1)
            # o-proj of window w-2 happens during window w
            return [(lambda ww=w - 2, qq=qc: emit_oproj_chunk(ww, qq))
                    for qc in range(4)] + [lambda ww=w - 2: emit_rs(ww)]

        for w, items in enumerate(by_window):
            fillers = window_fillers(w)
            n_items = len(items)
            emit_scores(items[0])
            fi = 0
            for i, it in enumerate(items):
                if i + 1 < n_items:
                    emit_scores(items[i + 1])
                # dispense fillers evenly across the window
                want = ((i + 1) * len(fillers)) // n_items
                while fi < want:
                    fillers[fi]()
                    fi += 1
                P = emit_expmask(it)
                emit_attnv(it, P)
            while fi < len(fillers):
                fillers[fi]()
                fi += 1
            emit_window_norm(w)
        # drain: o-proj + RS for the last two windows
        for w in (QB - 2, QB - 1):
            for qc in range(4):
                emit_oproj_chunk(w, qc)
            emit_rs(w)

    nc.compile()
    return nc


_PROGRAM_CACHE = {}


def _classify_mask(attention_mask):
    """Shared 16x16 block classification + per-batch mixed tile data."""
    m = np.asarray(attention_mask, np.float32)  # [B, 1, S, S]
    cls = [[ONES] * KC for _ in range(KC)]
    tiles = {}
    tile_data = [[], []]
    for kc in range(KC):
        for qc in range(KC):
            subs = [m[b, 0, qc * 128:(qc + 1) * 128,
                      kc * 128:(kc + 1) * 128] for b in range(B)]
            if all(np.all(s < -30.0) for s in subs):
                cls[kc][qc] = SKIP
            elif all(np.all(np.abs(s) < 1e-6) for s in subs):
                cls[kc][qc] = ONES
            else:
                es = [np.exp(np.minimum(s.T, 30.0)).astype(np.float32)
                      for s in subs]
                key = tuple(e.tobytes() for e in es)
                if key not in tiles:
                    tiles[key] = len(tiles) + 1
                    tile_data[0].append(es[0])
                    tile_data[1].append(es[1])
                cls[kc][qc] = tiles[key]
    return cls, tile_data


def _get_program(cls_map):
    key = tuple(tuple(r) for r in cls_map)
    if key not in _PROGRAM_CACHE:
        _PROGRAM_CACHE[key] = build_program(cls_map)
    return _PROGRAM_CACHE[key]


def make_in_maps(hidden_states, attention_mask, position_ids, cos, sin,
                 Wq, Wk, Wv, Wo):
    import ml_dtypes
    bf16 = ml_dtypes.bfloat16
    hidden_states = np.asarray(hidden_states, np.float32)
    position_ids = np.asarray(position_ids)
    cos = np.asarray(cos, np.float32)
    sin = np.asarray(sin, np.float32)
    Wq = np.asarray(Wq, np.float32) * SCALE
    Wk = np.asarray(Wk, np.float32)
    Wv = np.asarray(Wv, np.float32)
    Wo = np.asarray(Wo, np.float32)

    cls_map, tile_data = _classify_mask(attention_mask)

    sel = np.zeros((128, 256), np.float32)
    for p in range(2):
        for mm in range(128):
            sel[32 * (2 * p + (mm >= 64)), 128 * p + mm] = 1.0

    per_batch = []
    for b in range(B):
        hsT_b = np.ascontiguousarray(hidden_states[b].T).astype(bf16)
        cos_b = cos[position_ids[b]]  # [S, 64]
        sin_b = sin[position_ids[b]]
        cosT = np.tile(cos_b.T, (2, 1)).astype(bf16)  # [128, S]
        sin64 = sin_b.T
        sh = np.empty_like(sin64)
        sh[0:32] = -sin64[0:32]
        sh[32:64] = sin64[32:64]
        sinT = np.tile(sh, (2, 1)).astype(bf16)
        if tile_data[b]:
            emt_b = np.concatenate(tile_data[b], axis=1).astype(bf16)
        else:
            emt_b = np.ones((128, 128), np.float32).astype(bf16)
        per_batch.append((hsT_b, cosT, sinT, np.ascontiguousarray(emt_b)))

    in_maps = []
    for c in range(N_CORES):
        g, r = c // 4, c % 4
        hsT_b, cosT, sinT, emt_b = per_batch[g]
        in_maps.append({
            "hsT": hsT_b, "cosk": cosT, "sink": sinT, "emt": emt_b,
            "wq": np.ascontiguousarray(Wq[:, 256 * r:256 * (r + 1)]).astype(bf16),
            "wk": np.ascontiguousarray(Wk[:, 256 * r:256 * (r + 1)]).astype(bf16),
            "wv": np.ascontiguousarray(Wv[:, 256 * r:256 * (r + 1)]).astype(bf16),
            "wo": np.ascontiguousarray(Wo[256 * r:256 * (r + 1), :]).astype(bf16),
            "sel4": sel,
        })
    return in_maps, cls_map


def run(inputs: dict, trace: bool = False):
    in_maps, cls_map = make_in_maps(**inputs)
    nc = _get_program(cls_map)
    res = run_bass_kernel_spmd(nc, in_maps, list(range(N_CORES)), trace=trace)
    out = np.empty((B, S, HID), np.float32)
    for c in range(N_CORES):
        g, r = c // 4, c % 4
        o = np.asarray(res.results[c]["out"], np.float32)  # [512, 1024]
        for w in range(QB):
            s0 = w * 512 + r * 128
            out[g, s0:s0 + 128, :] = o[w * 128:(w + 1) * 128, :]
    return out, res


def kernel(**inputs) -> np.ndarray:
    out, _ = run(inputs, trace=False)
    return out


# revision 7
# speedup vs baseline: 1.2462x; 1.2462x over previous
"""Multi-head attention (RoPE + masked softmax) on 8 TRN2 NeuronCores.

Sharding: 2 batch groups x 4 cores; each core computes 4 of the 16 heads
(q/k/v projection column slices) for the full sequence of its batch, then a
per-window AllGather of the normalized attention outputs over the 4-core
group; each core o-projects one 128-row seq sub-shard per window with the
full Wo (host reassembles).

The attention mask is classified host-side into 128x128 blocks
(skip / identity / mixed) shared across both batches, and the program is
compiled specialized to that block map: fully-masked blocks are never
computed, identity blocks skip the mask multiply, and mixed blocks multiply
a per-block exp(mask) tile. Causal masks dedupe to a single mixed tile.

Schedule notes: inputs are DMA'd on the sync queue in consumption order
(packed one-trigger weight tiles) so the first projection matmul starts
~3us in; projection half 1 and the previous window's gather/o-proj are
dispensed as filler between attention items so the PE stays continuously
busy (p-state ramp). RoPE: one psum->sbuf copy (scalar/vector), a DMA
partition-shuffle for rotate-half (sin table pre-shuffled/signed host-side),
then the three multiplies/adds split between gpsimd (sbuf-only engine) and
vector.
"""

from contextlib import ExitStack

import numpy as np

import concourse.bass as bass
import concourse.tile as tile
from concourse import bacc, mybir
from concourse.alu_op_type import AluOpType
from concourse.bass_utils import run_bass_kernel_spmd

AF = mybir.ActivationFunctionType
F32 = mybir.dt.float32
F32R = mybir.dt.float32r
F16 = mybir.dt.float16
BF16 = mybir.dt.bfloat16

B, S, HID, NH, HD = 2, 2048, 1024, 16, 64
SCALE = 1.0 / np.sqrt(HD)
N_CORES = 8
HPC = 4          # heads per core
HC = HID // 128  # hidden 128-chunks (8)
KC = S // 128    # key 128-chunks (16)
QB = S // 512    # query windows of 512 (4)
GROUPS = [[0, 1, 2, 3], [4, 5, 6, 7]]

SKIP = -1
ONES = 0


def _runs(labels):
    """Maximal runs of non-skip entries: [(start, end)]."""
    out = []
    i = 0
    n = len(labels)
    while i < n:
        if labels[i] == SKIP:
            i += 1
            continue
        j = i
        while j < n and labels[j] != SKIP:
            j += 1
        out.append((i, j))
        i = j
    return out


def build_program(cls_map):
    """cls_map[kc][qc] over the 16x16 grid of (key, query) 128x128 blocks:
    SKIP (-1), ONES (0), or 1-based mixed-tile index."""
    n_mixed = max((v for row in cls_map for v in row if v > 0), default=0)

    nc = bacc.Bacc("TRN2", target_bir_lowering=False, debug=False,
                   num_devices=N_CORES)

    hsT = nc.dram_tensor("hsT", [HID, S], BF16, kind="ExternalInput").ap()
    cosk = nc.dram_tensor("cosk", [128, S], BF16, kind="ExternalInput").ap()
    sink = nc.dram_tensor("sink", [128, S], BF16, kind="ExternalInput").ap()
    # weights packed host-side as [128, HC*256] (hc along free dim)
    wq = nc.dram_tensor("wq", [128, HC * 256], BF16, kind="ExternalInput").ap()
    wk = nc.dram_tensor("wk", [128, HC * 256], BF16, kind="ExternalInput").ap()
    wv = nc.dram_tensor("wv", [128, HC * 256], BF16, kind="ExternalInput").ap()
    wo = nc.dram_tensor("wo", [HID, HID], BF16, kind="ExternalInput").ap()
    emt = nc.dram_tensor("emt", [128, max(n_mixed, 1) * 128], BF16,
                         kind="ExternalInput").ap()
    sel4 = nc.dram_tensor("sel4", [128, 256], F32R, kind="ExternalInput").ap()
    roff_t = nc.dram_tensor("roff", [1, 1], mybir.dt.uint32,
                            kind="ExternalInput").ap()
    out = nc.dram_tensor("out", [QB * 128, HID], F16, kind="ExternalOutput").ap()

    with tile.TileContext(nc) as tc, ExitStack() as top:
        res = top.enter_context(tc.tile_pool(name="res", bufs=1))
        dram = top.enter_context(tc.tile_pool(name="dram", bufs=1, space="DRAM"))

        # ---- warm-up collective: first thing on the gpsimd queue ---------
        ag_wu_in = dram.tile([8, 8], BF16, name="agwui")
        ag_wu_out = dram.tile([32, 8], BF16, name="agwuo")
        nc.gpsimd.collective_compute(
            "AllGather", AluOpType.bypass, replica_groups=GROUPS,
            ins=[ag_wu_in[:].opt()], outs=[ag_wu_out[:].opt()])

        # ---- resident tiles + input DMAs (consumption order, sync q) -----
        def packed_w(nm, src):
            t = res.tile([128, HC * 256], BF16, tag=nm, name=nm)
            nc.sync.dma_start(t[:], src[:])
            return t

        def rope_half(nm, src, half):
            t = res.tile([128, 1024], BF16, tag=f"{nm}{half}",
                         name=f"{nm}{half}")
            nc.sync.dma_start(t[:], src[:, half * 1024:(half + 1) * 1024])
            return t

        hsT_sb = [[None, None] for _ in range(HC)]

        def hsT_half(half):
            for hc in range(HC):
                t = res.tile([128, 1024], BF16, tag=f"hsT{hc}_{half}",
                             name=f"hsT{hc}_{half}")
                nc.sync.dma_start(t[:], hsT[hc * 128:(hc + 1) * 128,
                                            half * 1024:(half + 1) * 1024])
                hsT_sb[hc][half] = t

        wk_sb = packed_w("wk", wk)
        hsT_half(0)
        cos_sb = [rope_half("cos", cosk, 0), None]
        sin_sb = [rope_half("sin", sink, 0), None]
        wq_sb = packed_w("wq", wq)
        wv_sb = packed_w("wv", wv)
        hsT_half(1)
        cos_sb[1] = rope_half("cos", cosk, 1)
        sin_sb[1] = rope_half("sin", sink, 1)
        emt_sb = res.tile([128, max(n_mixed, 1) * 128], BF16, tag="emt")
        nc.sync.dma_start(emt_sb[:], emt[:])
        sel_sb = res.tile([128, 256], F32R, tag="sel")
        nc.sync.dma_start(sel_sb[:], sel4[:])
        wo_sb = []
        for hc in range(HC):
            t = res.tile([128, HID], BF16, tag=f"wo{hc}", name=f"wo{hc}")
            nc.sync.dma_start(t[:], wo[hc * 128:(hc + 1) * 128, :])
            wo_sb.append(t)

        def wslice(t, hc, c0, cn):
            return t[:, hc * 256 + c0: hc * 256 + c0 + cn]

        # V_aug: per key-chunk, 4 heads x (64 cols + ones col)
        v_sb = [res.tile([128, HPC * 65], BF16, tag=f"v{kc}", name=f"v{kc}")
                for kc in range(KC)]
        for kc in range(KC):
            v3 = v_sb[kc][:].rearrange("p (h c) -> p h c", h=HPC)
            nc.gpsimd.memset(v3[:, :, 64], 1.0)
        kt_sb = [res.tile([128, S], BF16, tag=f"kt{p}", name=f"kt{p}")
                 for p in range(2)]
        qt_sb = [res.tile([128, S], BF16, tag=f"qt{p}", name=f"qt{p}")
                 for p in range(2)]
        # per-window unnormalized / normalized attention outputs
        acc_w = [[res.tile([128, 512], BF16, tag=f"acc{w}_{p}",
                           name=f"acc{w}_{p}")
                  for p in range(2)] for w in range(QB)]
        acc2_w = [[res.tile([128, 512], BF16, tag=f"acc2_{w}_{p}",
                            name=f"acc2_{w}_{p}")
                   for p in range(2)] for w in range(QB)]
        # den rows live at partition 32*h (DVE writes must start 32-aligned)
        den_w = [res.tile([128, 512], F32, tag=f"den{w}", name=f"den{w}")
                 for w in range(QB)]
        recip_w = [res.tile([128, 512], F32R, tag=f"recip{w}",
                            name=f"recip{w}")
                   for w in range(QB)]
        for w in range(QB):
            nc.gpsimd.memset(den_w[w][:], 1.0)

        # DRAM bounce buffers for the per-window AllGather of attn outputs
        ag_in = [dram.tile([256, 512], BF16, name=f"agi{w}") for w in range(QB)]
        ag_out = [dram.tile([1024, 512], BF16, name=f"ago{w}")
                  for w in range(QB)]

        pss = top.enter_context(tc.tile_pool(name="pss", bufs=2, space="PSUM"))
        psa = top.enter_context(tc.tile_pool(name="psa", bufs=2, space="PSUM"))
        pbo = top.enter_context(tc.tile_pool(name="pbo", bufs=2, space="PSUM"))
        pp = top.enter_context(tc.tile_pool(name="pp", bufs=5))
        outp = top.enter_context(tc.tile_pool(name="outp", bufs=3))
        ropep = top.enter_context(tc.tile_pool(name="ropep", bufs=6))

        _rreg = nc.gpsimd.alloc_register("roff_reg")
        nc.gpsimd.reg_load(_rreg, roff_t[0:1, 0:1])
        roff = nc.gpsimd.snap(_rreg, donate=True, min_val=0, max_val=384)

        # ---- Q/K/V projections + RoPE ------------------------------------
        def rope_apply(ps2, dst_tile, half, copy_eng):
            """dst[:, half*1024:+1024] = rope(ps2) over a full 1024-col chain.
            One psum->sbuf copy (scalar or vector), a DMA partition-shuffle
            for rotate-half, then t2 on vector, t1+add on gpsimd (sin table
            pre-shuffled/signed host-side)."""
            g0 = half * 1024
            cs = cos_sb[half]
            sn = sin_sb[half]
            with nc.allow_low_precision(reason="bf16 rope"):
                kraw = ropep.tile([128, 1024], BF16, tag="kraw", name="kraw")
                if copy_eng is nc.scalar:
                    nc.scalar.copy(kraw[:], ps2[:])
                else:
                    nc.vector.tensor_copy(kraw[:], ps2[:])
                krot = ropep.tile([128, 1024], BF16, tag="krot", name="krot")
                for hb in (0, 64):
                    nc.sync.dma_start(krot[hb:hb + 32, :],
                                      kraw[hb + 32:hb + 64, :])
                    nc.sync.dma_start(krot[hb + 32:hb + 64, :],
                                      kraw[hb:hb + 32, :])
                t1 = ropep.tile([128, 1024], BF16, tag="t1", name="t1")
                nc.gpsimd.tensor_tensor(t1[:], kraw[:], cs[:],
                                        AluOpType.mult)
                nc.vector.tensor_tensor(krot[:], krot[:], sn[:],
                                        AluOpType.mult)
                nc.gpsimd.tensor_tensor(
                    dst_tile[:, g0:g0 + 1024], t1[:], krot[:], AluOpType.add)

        def emit_kq_chain(w_sb, dst, p, half, copy_eng):
            """One [128, 1024] projection chain + rope for both 512-windows."""
            c0 = p * 128
            ps2 = pss.tile([128, 1024], F32, tag="pss", name="pskq")
            for hc in range(HC):
                for j in range(2):
                    nc.tensor.matmul(
                        ps2[:, j * 512:(j + 1) * 512],
                        wslice(w_sb, hc, c0, 128),
                        hsT_sb[hc][half][:, j * 512:(j + 1) * 512],
                        start=(hc == 0), stop=(hc == HC - 1))
            rope_apply(ps2, dst[p], half, copy_eng)

        def emit_v(kc):
            psf = pbo.tile([128, 512], F32, tag="pbo", name="psvf")
            ps = psf[:, 0:256]
            half, cc = kc // 8, (kc % 8) * 128
            for hc in range(HC):
                nc.tensor.matmul(
                    ps, hsT_sb[hc][half][:, cc:cc + 128],
                    wslice(wv_sb, hc, 0, 256),
                    start=(hc == 0), stop=(hc == HC - 1))
            v3 = v_sb[kc][:].rearrange("p (h c) -> p h c", h=HPC)
            ps3 = ps.rearrange("p (h c) -> p h c", h=HPC)
            with nc.allow_low_precision(reason="bf16 V"):
                nc.vector.tensor_copy(v3[:, :, 0:64], ps3[:])

        def proj_half_units(half):
            """Filler units (thunks) for projecting one 1024-col half."""
            units = []
            units.append(lambda h=half: emit_kq_chain(wk_sb, kt_sb, 0, h,
                                                      nc.scalar))
            units.append(lambda h=half: emit_kq_chain(wk_sb, kt_sb, 1, h,
                                                      nc.vector))
            units.append(lambda h=half: emit_v(8 * half + 0))
            units.append(lambda h=half: emit_v(8 * half + 1))
            units.append(lambda h=half: emit_kq_chain(wq_sb, qt_sb, 0, h,
                                                      nc.scalar))
            units.append(lambda h=half: emit_v(8 * half + 2))
            units.append(lambda h=half: emit_v(8 * half + 3))
            units.append(lambda h=half: emit_kq_chain(wq_sb, qt_sb, 1, h,
                                                      nc.vector))
            for kc in range(8 * half + 4, 8 * half + 8):
                units.append(lambda k=kc: emit_v(k))
            return units

        # ---- attention items ---------------------------------------------
        # item = (h, qb, kcs(list), runs(per kc), start, stop)
        by_window = []
        for qb in range(QB):
            items = []
            for h in range(HPC):
                parts = [kc for kc in range(KC)
                         if any(cls_map[kc][4 * qb + j] != SKIP
                                for j in range(4))]
                assert parts, f"no participating key blocks for window {qb}"
                for i2 in range(0, len(parts), 2):
                    kcs = parts[i2:i2 + 2]
                    runs = [_runs([cls_map[kc][4 * qb + j] for j in range(4)])
                            for kc in kcs]
                    items.append([h, qb, kcs, runs,
                                  i2 == 0, i2 + 2 >= len(parts)])
            by_window.append(items)

        psa_t = {}
        ps_of_item = {}

        def emit_scores(it):
            h, qb, kcs, runs, start, stop = it
            p, hb = h // 2, (h % 2) * 64
            ps_s = pss.tile([128, 1024], F32, tag="pss")
            for j, kc in enumerate(kcs):
                for r0, r1 in runs[j]:
                    nc.tensor.matmul(
                        ps_s[:, j * 512 + r0 * 128: j * 512 + r1 * 128],
                        kt_sb[p][hb:hb + 64, kc * 128:(kc + 1) * 128],
                        qt_sb[p][hb:hb + 64,
                                 qb * 512 + r0 * 128: qb * 512 + r1 * 128],
                        start=True, stop=True)
            ps_of_item[id(it)] = ps_s

        def emit_expmask(it):
            h, qb, kcs, runs, start, stop = it
            ps_s = ps_of_item.pop(id(it))
            P = pp.tile([128, 1024], BF16, tag="pp")
            if len(kcs) == 2 and runs[0] == runs[1] == [(0, 4)]:
                nc.scalar.activation(P[:], ps_s[:], AF.Exp)
            else:
                for j in range(len(kcs)):
                    for r0, r1 in runs[j]:
                        nc.scalar.activation(
                            P[:, j * 512 + r0 * 128: j * 512 + r1 * 128],
                            ps_s[:, j * 512 + r0 * 128: j * 512 + r1 * 128],
                            AF.Exp)
            for j, kc in enumerate(kcs):
                labels = [cls_map[kc][4 * qb + jj] for jj in range(4)]
                if start and j == 0 and runs[j] != [(0, 4)]:
                    # first accumulation must initialize all 512 cols
                    zr = [jj for jj in range(4) if labels[jj] == SKIP]
                    jj = 0
                    while jj < len(zr):
                        j2 = jj
                        while j2 + 1 < len(zr) and zr[j2 + 1] == zr[j2] + 1:
                            j2 += 1
                        nc.gpsimd.memset(
                            P[:, j * 512 + zr[jj] * 128:
                              j * 512 + (zr[j2] + 1) * 128],
                            0.0)
                        jj = j2 + 1
                for jj in range(4):
                    mix = labels[jj]
                    if mix > 0:
                        sl = P[:, j * 512 + jj * 128: j * 512 + (jj + 1) * 128]
                        with nc.allow_low_precision(reason="bf16 mask"):
                            nc.gpsimd.tensor_tensor(
                                sl, sl,
                                emt_sb[:, (mix - 1) * 128: mix * 128],
                                AluOpType.mult)
            return P

        def emit_attnv(it, P):
            h, qb, kcs, runs, start, stop = it
            if start:
                psa_t[(h, qb)] = psa.tile([128, 512], F32, tag="psa",
                                          name="psat")
            pa = psa_t[(h, qb)]
            for j, kc in enumerate(kcs):
                first = start and j == 0
                last = stop and j == len(kcs) - 1
                if first:
                    nc.tensor.matmul(
                        pa[0:65, :], v_sb[kc][:, h * 65: h * 65 + 65],
                        P[:, j * 512:(j + 1) * 512],
                        start=True, stop=last)
                else:
                    for r0, r1 in runs[j]:
                        nc.tensor.matmul(
                            pa[0:65, r0 * 128:r1 * 128],
                            v_sb[kc][:, h * 65: h * 65 + 65],
                            P[:, j * 512 + r0 * 128: j * 512 + r1 * 128],
                            start=False,
                            stop=last and (r0, r1) == runs[j][-1])
            if stop:
                p, hb = h // 2, (h % 2) * 64
                nc.vector.tensor_copy(den_w[qb][32 * h:32 * h + 1, :],
                                      pa[64:65, :])
                with nc.allow_low_precision(reason="bf16 attn accum"):
                    nc.vector.tensor_copy(acc_w[qb][p][hb:hb + 64, :],
                                          pa[0:64, :])
                del psa_t[(h, qb)]

        def emit_window_norm(w):
            """normalize acc -> acc2, ship to ag_in, trigger AllGather."""
            from concourse.dve_ops import (
                RECIP_APPROX_FAST_CONSTS,
                RECIPROCAL_APPROX_FAST,
            )
            c = RECIP_APPROX_FAST_CONSTS
            with nc.allow_low_precision(reason="f32r reciprocal"):
                nc.vector._custom_dve(
                    RECIPROCAL_APPROX_FAST, out=recip_w[w][:],
                    in0=den_w[w][:], s0=c["s0"], s1=c["s1"], imm2=c["imm2"])
            for p in range(2):
                ps_bc = pbo.tile([128, 512], F32, tag="pbo", name="psbc")
                nc.tensor.matmul(ps_bc[:],
                                 sel_sb[:, p * 128:(p + 1) * 128],
                                 recip_w[w][:], start=True, stop=True)
                with nc.allow_low_precision(reason="bf16 attn weights"):
                    nc.vector.tensor_tensor(
                        acc2_w[w][p][:], acc_w[w][p][:], ps_bc[:],
                        AluOpType.mult)
                nc.sync.dma_start(ag_in[w][p * 128:(p + 1) * 128, :],
                                  acc2_w[w][p][:])
            nc.gpsimd.collective_compute(
                "AllGather", AluOpType.bypass, replica_groups=GROUPS,
                ins=[ag_in[w][:].opt()], outs=[ag_out[w][:].opt()])

        def emit_tail_b(w):
            """gather this core's 128 q rows of window w, o-proj, write out."""
            aot = outp.tile([128, HC * 128], BF16, tag="aot", name="aot")
            ao3 = aot[:].rearrange("b (a c) -> b a c", a=HC)
            src = ag_out[w][:].rearrange("(a b) c -> b a c", b=128)
            nc.gpsimd.dma_start(ao3[:, :, :],
                                src[:, :, bass.ds(roff, 128)])
            for nn in range(2):
                ps = pbo.tile([128, 512], F32, tag="pbo", name="pso")
                for kk in range(HC):
                    nc.tensor.matmul(
                        ps[:], ao3[:, kk, :],
                        wo_sb[kk][:, nn * 512:(nn + 1) * 512],
                        start=(kk == 0), stop=(kk == HC - 1))
                t_o = outp.tile([128, 512], F16, tag="tout")
                with nc.allow_low_precision(reason="f16 out"):
                    nc.vector.tensor_copy(t_o[:], ps[:])
                nc.sync.dma_start(
                    out[w * 128:(w + 1) * 128,
                        nn * 512:(nn + 1) * 512], t_o[:])

        # ---- schedule -----------------------------------------------------
        # proj half 0 up front; proj half 1 (during w0/w1) and the previous
        # window's gather+o-proj dispensed as filler between attention items.
        h0_units = proj_half_units(0)
        for unit in h0_units[:10]:
            unit()

        def window_fillers(w):
            if w == 0:
                return h0_units[10:]
            if w == 1:
                return proj_half_units(1)
            # tail (gather + o-proj) of window w-2 happens during window w
            return [lambda ww=w - 2: emit_tail_b(ww)]

        for w, items in enumerate(by_window):
            fillers = window_fillers(w)
            n_items = len(items)
            emit_scores(items[0])
            fi = 0
            for i, it in enumerate(items):
                if i + 1 < n_items:
                    emit_scores(items[i + 1])
                # dispense fillers evenly across the window
                want = ((i + 1) * len(fillers)) // n_items
                while fi < want:
                    fillers[fi]()
                    fi += 1
                P = emit_expmask(it)
                emit_attnv(it, P)
            while fi < len(fillers):
                fillers[fi]()
                fi += 1
            emit_window_norm(w)
        # drain: gather + o-proj for the last two windows
        emit_tail_b(QB - 2)
        emit_tail_b(QB - 1)

    nc.compile()
    return nc


_PROGRAM_CACHE = {}


def _classify_mask(attention_mask):
    """Shared 16x16 block classification + per-batch mixed tile data."""
    m = np.asarray(attention_mask, np.float32)  # [B, 1, S, S]
    cls = [[ONES] * KC for _ in range(KC)]
    tiles = {}
    tile_data = [[], []]
    for kc in range(KC):
        for qc in range(KC):
            subs = [m[b, 0, qc * 128:(qc + 1) * 128,
                      kc * 128:(kc + 1) * 128] for b in range(B)]
            if all(np.all(s < -30.0) for s in subs):
                cls[kc][qc] = SKIP
            elif all(np.all(np.abs(s) < 1e-6) for s in subs):
                cls[kc][qc] = ONES
            else:
                es = [np.exp(np.minimum(s.T, 30.0)).astype(np.float32)
                      for s in subs]
                key = tuple(e.tobytes() for e in es)
                if key not in tiles:
                    tiles[key] = len(tiles) + 1
                    tile_data[0].append(es[0])
                    tile_data[1].append(es[1])
                cls[kc][qc] = tiles[key]
    return cls, tile_data


def _get_program(cls_map):
    key = tuple(tuple(r) for r in cls_map)
    if key not in _PROGRAM_CACHE:
        _PROGRAM_CACHE[key] = build_program(cls_map)
    return _PROGRAM_CACHE[key]


def make_in_maps(hidden_states, attention_mask, position_ids, cos, sin,
                 Wq, Wk, Wv, Wo):
    import ml_dtypes
    bf16 = ml_dtypes.bfloat16
    hidden_states = np.asarray(hidden_states, np.float32)
    position_ids = np.asarray(position_ids)
    cos = np.asarray(cos, np.float32)
    sin = np.asarray(sin, np.float32)
    Wq = np.asarray(Wq, np.float32) * SCALE
    Wk = np.asarray(Wk, np.float32)
    Wv = np.asarray(Wv, np.float32)
    Wo = np.asarray(Wo, np.float32)

    cls_map, tile_data = _classify_mask(attention_mask)

    sel = np.zeros((128, 256), np.float32)
    for p in range(2):
        for mm in range(128):
            sel[32 * (2 * p + (mm >= 64)), 128 * p + mm] = 1.0

    wo_full = np.ascontiguousarray(Wo).astype(bf16)
    per_batch = []
    for b in range(B):
        hsT_b = np.ascontiguousarray(hidden_states[b].T).astype(bf16)
        cos_b = cos[position_ids[b]]  # [S, 64]
        sin_b = sin[position_ids[b]]
        cosT = np.tile(cos_b.T, (2, 1)).astype(bf16)  # [128, S]
        sin64 = sin_b.T
        sh = np.empty_like(sin64)
        sh[0:32] = -sin64[0:32]
        sh[32:64] = sin64[32:64]
        sinT = np.tile(sh, (2, 1)).astype(bf16)
        if tile_data[b]:
            emt_b = np.concatenate(tile_data[b], axis=1).astype(bf16)
        else:
            emt_b = np.ones((128, 128), np.float32).astype(bf16)
        per_batch.append((hsT_b, cosT, sinT, np.ascontiguousarray(emt_b)))

    def pack_w(W, r):
        """[HID, 256] column slice -> [128, HC*256] with hc along free."""
        Wl = np.ascontiguousarray(W[:, 256 * r:256 * (r + 1)])
        return np.ascontiguousarray(
            Wl.reshape(HC, 128, 256).transpose(1, 0, 2).reshape(128, HC * 256)
        ).astype(bf16)

    in_maps = []
    for c in range(N_CORES):
        g, r = c // 4, c % 4
        hsT_b, cosT, sinT, emt_b = per_batch[g]
        in_maps.append({
            "hsT": hsT_b, "cosk": cosT, "sink": sinT, "emt": emt_b,
            "wq": pack_w(Wq, r), "wk": pack_w(Wk, r), "wv": pack_w(Wv, r),
            "wo": wo_full, "sel4": sel,
            "roff": np.array([[r * 128]], np.uint32),
        })
    return in_maps, cls_map


def run(inputs: dict, trace: bool = False):
    in_maps, cls_map = make_in_maps(**inputs)
    nc = _get_program(cls_map)
    res = run_bass_kernel_spmd(nc, in_maps, list(range(N_CORES)), trace=trace)
    out = np.empty((B, S, HID), np.float32)
    for c in range(N_CORES):
        g, r = c // 4, c % 4
        o = np.asarray(res.results[c]["out"], np.float32)  # [512, 1024]
        for w in range(QB):
            s0 = w * 512 + r * 128
            out[g, s0:s0 + 128, :] = o[w * 128:(w + 1) * 128, :]
    return out, res


def kernel(**inputs) -> np.ndarray:
    out, _ = run(inputs, trace=False)
    return out
